# revision 18
# baseline (speedup 1.0000x reference)
"""Trainium2 Bass kernel for nn_ContextualEncoder2 (5-step GRU over buoys).

fp8 DoubleRow formulation (data-parallel, 2048 buoys/core, blocks of 1024):

  h_t = hbar_t(id) + d_t  with hbar_t the obs=0 GRU trajectory of the id's
  embedding (100 ids, host-precomputed). All W.h contractions become
    W.h = [W.hbar](id)  (exact per-id table, applied via a onehot matmul)
        + Q8(W).Q8(d)   (residual d is ~10x smaller than h, so 1-term fp8
                         quantization error is ~10x smaller too)
  Every matmul is a float8e4 DoubleRow (2 contraction rows/cycle):
    * tables: stationary [A|B] (2-term fp8 split of 16*T) x moving [oh|oh]
      (onehot columns scaled by 16; PSUM accumulates 256x the math value)
    * obs: stationary [[A;A] | [B;B]] x moving [opack|opack] where opack
      stacks fp8 hi/lo parts of 16*obs on partitions 0:64 / 64:128 - a full
      (A+B)(hi+lo) product in ONE DoubleRow matmul
    * d: stationary fp8(16W) chunk pairs x moving [d_2c|d_2c+1], d = fp8(16d)
  Gate math (1/256 scale folded into ACT), h kept as 16*h in fp16 for the
  elementwise ops; d produced directly in fp8 by a DVE STT reading the
  -256*hbar broadcast PSUM (bc tables applied per position).
"""
import numpy as np
import ml_dtypes

import concourse.bass as bass
import concourse.mybir as mybir
import concourse.tile as tile
from concourse import bacc
from concourse.bass_utils import run_bass_kernel_spmd

F32 = mybir.dt.float32
F16 = mybir.dt.float16
F8 = mybir.dt.float8e4
E4M3 = ml_dtypes.float8_e4m3
AF = mybir.ActivationFunctionType
OP = mybir.AluOpType
DR = mybir.MatmulPerfMode.DoubleRow

N_CORES = 8
NUM_BUOYS = 16384
H = 1024
NEMB = 100
FCH = 8          # 1024/128 gate-feature tiles per gate
KP = 4           # d contraction pairs (1024 / 256)
NT = 512         # moving free tile width (one PSUM bank)


def build(nbuoy=2048, blk=1024):
    assert nbuoy % blk == 0 and blk % NT == 0
    nblk = nbuoy // blk
    J = blk // NT

    nc = bacc.Bacc("TRN2", target_bir_lowering=False, debug=False)

    # --- DRAM parameters -------------------------------------------------
    # d-part weights: [m, p, c, i, j] = fp8(16*W)[128m+j, 128(2c+i)+p]
    whh = nc.declare_dram_parameter("whh", [24, 128, 4, 2, 128], F8, isOutput=False)
    wih = nc.declare_dram_parameter("wih", [24, 128, 4, 2, 128], F8, isOutput=False)
    wsum = nc.declare_dram_parameter("wsum", [16, 128, 4, 2, 128], F8,
                                     isOutput=False)
    # obs stationaries ([A;A] | [B;B] packs)
    wobs = nc.declare_dram_parameter("wobs", [24, 128, 2, 128], F8, isOutput=False)
    wobs45 = nc.declare_dram_parameter("wobs45", [24, 128, 2, 128], F8,
                                       isOutput=False)
    # tables: [tile, p(=id), slot(A|B), j]
    tabs = {}
    for name, nt_ in [("ts1", 24), ("t2rz", 16), ("t2n", 8), ("t3rz", 16),
                      ("t3n", 8), ("t4rz", 16), ("t4n", 8), ("t4gi", 8),
                      ("t5rz", 16), ("t5n", 8), ("t5gi", 8)]:
        tabs[name] = nc.declare_dram_parameter(name, [nt_, 128, 2, 128], F8,
                                               isOutput=False)
    bc = nc.declare_dram_parameter("bc", [4, 8, 128, 2, 128], F8, isOutput=False)
    opk = nc.declare_dram_parameter("opk", [3, 128, nbuoy], F8, isOutput=False)
    oh = nc.declare_dram_parameter("oh", [128, nbuoy], F8, isOutput=False)
    bsum = nc.declare_dram_parameter("bsum", [128, 24], F32, isOutput=False)
    bih = nc.declare_dram_parameter("bih", [128, 24], F32, isOutput=False)
    bhh256 = nc.declare_dram_parameter("bhh256", [128, 24], F32, isOutput=False)
    out_t = nc.declare_dram_parameter("out_t", [FCH, 128, nbuoy], F32,
                                      isOutput=True)

    whh_ap, wih_ap, wsum_ap = whh.ap(), wih.ap(), wsum.ap()
    out_ap = out_t.ap()

    with tile.TileContext(nc) as tc:
        with (
            tc.tile_pool(name="const", bufs=1) as cpool,
            tc.tile_pool(name="obsl", bufs=2) as opool,
            tc.tile_pool(name="htiles", bufs=1) as hpool,
            tc.tile_pool(name="work", bufs=2) as wpool,
            tc.tile_pool(name="wstr", bufs=16) as spool,
            tc.tile_pool(name="psB", bufs=2, space="PSUM") as psB,
            tc.tile_pool(name="psC", bufs=1, space="PSUM") as psC,
        ):
            # ---- resident constants -----------------------------------
            def ctile(shape, tag):
                return cpool.tile(shape, F8, tag=tag, name=tag)

            whh_sb = []
            for m in range(24):
                t = ctile([128, 4, 2, 128], f"whh{m}")
                nc.sync.dma_start(t[:], whh_ap[m])
                whh_sb.append(t)

            def load_tab(name, nt_):
                ts = []
                ap = tabs[name].ap()
                for i in range(nt_):
                    t = ctile([128, 2, 128], f"{name}{i}")
                    nc.sync.dma_start(t[:], ap[i])
                    ts.append(t)
                return ts

            ts1_sb = load_tab("ts1", 24)
            t2rz_sb = load_tab("t2rz", 16)
            t2n_sb = load_tab("t2n", 8)
            t3rz_sb = load_tab("t3rz", 16)
            t3n_sb = load_tab("t3n", 8)
            t4rz_sb = load_tab("t4rz", 16)
            t4n_sb = load_tab("t4n", 8)
            t4gi_sb = load_tab("t4gi", 8)
            t5rz_sb = load_tab("t5rz", 16)
            t5n_sb = load_tab("t5n", 8)
            t5gi_sb = load_tab("t5gi", 8)

            wobs_sb, wobs45_sb = [], []
            for m in range(24):
                t = ctile([128, 2, 128], f"wo{m}")
                nc.sync.dma_start(t[:], wobs.ap()[m])
                wobs_sb.append(t)
                t = ctile([128, 2, 128], f"wo45_{m}")
                nc.sync.dma_start(t[:], wobs45.ap()[m])
                wobs45_sb.append(t)
            bc_sb = [[None] * 8 for _ in range(4)]
            for s in range(4):
                for c in range(8):
                    t = ctile([128, 2, 128], f"bc{s}_{c}")
                    nc.sync.dma_start(t[:], bc.ap()[s][c])
                    bc_sb[s][c] = t

            bsum_sb = cpool.tile([128, 24], F32, tag="bsum", name="bsum")
            nc.sync.dma_start(bsum_sb[:], bsum.ap())
            bih_sb = cpool.tile([128, 24], F32, tag="bih", name="bih")
            nc.sync.dma_start(bih_sb[:], bih.ap())
            bhh_sb = cpool.tile([128, 24], F32, tag="bhh256", name="bhh256")
            nc.sync.dma_start(bhh_sb[:], bhh256.ap())

            # ---- per-block state --------------------------------------
            st = {b: {"h": {}, "obs": {}, "d": {}, "d1": {}} for b in range(nblk)}
            ps_tags = {"pr": "pr", "pz": "pz", "pgh": "pgh", "pg": "pg"}

            def init_block(b):
                cb = b * blk
                o = st[b]["obs"]
                for t_ in range(3):
                    tl = opool.tile([128, 2, blk], F8, tag=f"op{t_}",
                                    name=f"op{t_}")
                    for i in range(2):
                        nc.sync.dma_start(tl[:, i, :], opk.ap()[t_][:, cb:cb + blk])
                    o[t_] = tl
                tl = opool.tile([128, 2, blk], F8, tag="oh", name="oht")
                for i in range(2):
                    nc.sync.dma_start(tl[:, i, :], oh.ap()[:, cb:cb + blk])
                o["oh"] = tl
                # d-pair tiles (fp8): families A (d1, kept for s4), B, C
                for fam in "ABC":
                    st[b]["d"][fam] = [
                        hpool.tile([128, 2, blk], F8, tag=f"d{fam}{c}",
                                   name=f"d{fam}{c}") for c in range(KP)
                    ]

            def accum(p, pairs):
                last = len(pairs) - 1
                for i, (l, r) in enumerate(pairs):
                    nc.tensor.matmul(p, l, r, start=(i == 0), stop=(i == last),
                                     perf_mode=DR)

            def gates_tail(b, s, f, jj, pr, pz, pgh, pg, bcp, hprev, fam):
                """Shared ACT/DVE tail; returns nothing (writes h/d/out)."""
                cb = b * blk
                c0, c1 = jj * NT, (jj + 1) * NT
                mr, mz, mn = f, 8 + f, 16 + f
                r = wpool.tile([128, NT], F16, tag="r", name="r")
                nc.scalar.activation(r[:], pr[:], AF.Sigmoid,
                                     bias=bsum_sb[:, mr:mr + 1], scale=1 / 256.)
                z = wpool.tile([128, NT], F16, tag="z", name="z")
                nc.scalar.activation(z[:], pz[:], AF.Sigmoid,
                                     bias=bsum_sb[:, mz:mz + 1], scale=1 / 256.)
                t2 = wpool.tile([128, NT], F32, tag="t2", name="t2")
                if s == 1:
                    # gh = 0: t2 = 256*(r*c_n) + pg
                    nc.vector.scalar_tensor_tensor(
                        t2[:], r[:], bhh_sb[:, mn:mn + 1], pg[:],
                        OP.mult, OP.add)
                else:
                    t1 = wpool.tile([128, NT], F32, tag="t1", name="t1")
                    nc.vector.scalar_tensor_tensor(
                        t1[:], pgh[:], bhh_sb[:, mn:mn + 1], r[:],
                        OP.add, OP.mult)
                    nc.vector.tensor_add(t2[:], t1[:], pg[:])
                n = wpool.tile([128, NT], F16, tag="n", name="n")
                nc.scalar.activation(n[:], t2[:], AF.Tanh,
                                     bias=bih_sb[:, mn:mn + 1], scale=1 / 256.)
                n16 = wpool.tile([128, NT], F16, tag="n16", name="n16")
                nc.vector.tensor_scalar_mul(n16[:], n[:], 16.0)
                u = wpool.tile([128, NT], F16, tag="u", name="u")
                if s == 1:
                    nc.vector.tensor_mul(u[:], z[:], n16[:])
                else:
                    nc.vector.tensor_sub(u[:], hprev[:], n16[:])
                e = wpool.tile([128, NT], F16, tag="e", name="e")
                if s > 1:
                    nc.vector.tensor_mul(e[:], z[:], u[:])
                if s < 5:
                    hn = hpool.tile([128, NT], F16, tag=f"h{fam}_{f}_{jj}",
                                    name="hn")
                    if s == 1:
                        nc.vector.tensor_sub(hn[:], n16[:], u[:])
                    else:
                        nc.vector.tensor_add(hn[:], n16[:], e[:])
                    st[b]["h"][(f, jj)] = hn
                    dfam = st[b]["d"]["A" if s == 1 else "BCB"[s - 2]]
                    nc.vector.scalar_tensor_tensor(
                        dfam[f // 2][:, f % 2, c0:c1], bcp[:], 1 / 16., hn[:],
                        OP.mult, OP.add)
                else:
                    ho = wpool.tile([128, NT], F32, tag="ho", name="ho")
                    nc.vector.scalar_tensor_tensor(
                        ho[:], e[:], 1 / 16., n[:], OP.mult, OP.add)
                    nc.sync.dma_start(out_ap[f][:, cb + c0:cb + c1], ho[:])

            def step1(b):
                obs = st[b]["obs"]
                hnew = {}
                with nc.named_scope(f"b{b}s1"):
                    for f in range(FCH):
                        mr, mz, mn = f, 8 + f, 16 + f
                        for jj in range(J):
                            c0, c1 = jj * NT, (jj + 1) * NT
                            ohs = obs["oh"][:, :, c0:c1]
                            ops = obs[0][:, :, c0:c1]
                            pr = psB.tile([128, NT], F32, tag="pr", name="pr")
                            accum(pr[:], [(ts1_sb[mr][:], ohs),
                                          (wobs_sb[mr][:], ops)])
                            pz = psB.tile([128, NT], F32, tag="pz", name="pz")
                            accum(pz[:], [(ts1_sb[mz][:], ohs),
                                          (wobs_sb[mz][:], ops)])
                            pg = psC.tile([128, NT], F32, tag="pg", name="pg")
                            accum(pg[:], [(ts1_sb[mn][:], ohs),
                                          (wobs_sb[mn][:], ops)])
                            bcp = psB.tile([128, NT], F32, tag="bc", name="bcp")
                            accum(bcp[:], [(bc_sb[0][f][:], ohs)])
                            gates_tail(b, 1, f, jj, pr, pz, None, pg, bcp,
                                       None, "A")
                st[b]["h1"] = dict(st[b]["h"])

            def prefetch(b, s):
                """DMA the streamed weight slices for step s one step early."""
                sl = []
                for f in range(FCH):
                    mr, mz, mn = f, 8 + f, 16 + f
                    row = []
                    srcs = ([wih_ap[mr], wih_ap[mz], wih_ap[mn]] if s == 4 else
                            [wsum_ap[mr], wsum_ap[mz], wih_ap[mn]])
                    for src in srcs:
                        t = spool.tile([128, 4, 2, 128], F8, tag="wsl",
                                       name="wsl")
                        nc.sync.dma_start(t[:], src)
                        row.append(t)
                    sl.append(row)
                st[b][f"w{s}"] = sl

            def stepn(b, s):
                obs = st[b]["obs"]
                hcur = st[b]["h"]
                st[b]["h"] = {}
                dprev = st[b]["d"]["ABC"[s - 2]] if s <= 4 else st[b]["d"]["B"]
                d1 = st[b]["d"]["A"]
                ot = obs[s - 1] if s <= 3 else obs[s - 3]
                wob = wobs_sb if s <= 3 else wobs45_sb
                trz = {2: t2rz_sb, 3: t3rz_sb, 4: t4rz_sb, 5: t5rz_sb}[s]
                tn = {2: t2n_sb, 3: t3n_sb, 4: t4n_sb, 5: t5n_sb}[s]
                if s == 3:
                    prefetch(b, 4)
                elif s == 4:
                    prefetch(b, 5)
                with nc.named_scope(f"b{b}s{s}"):
                    for f in range(FCH):
                        mr, mz, mn = f, 8 + f, 16 + f
                        if s == 4:
                            vih = st[b]["w4"][f]
                        elif s == 5:
                            vs = st[b]["w5"][f]
                        for jj in range(J):
                            c0, c1 = jj * NT, (jj + 1) * NT
                            ohs = obs["oh"][:, :, c0:c1]
                            ops = ot[:, :, c0:c1]
                            dmov = [dprev[c][:, :, c0:c1] for c in range(KP)]
                            d1mov = [d1[c][:, :, c0:c1] for c in range(KP)]

                            pr = psB.tile([128, NT], F32, tag="pr", name="pr")
                            pairs = [(trz[mr][:], ohs)]
                            if s == 5:
                                pairs += [(vs[0][:, c], dmov[c])
                                          for c in range(KP)]
                            else:
                                pairs += [(whh_sb[mr][:, c], dmov[c])
                                          for c in range(KP)]
                            if s == 4:
                                pairs += [(vih[0][:, c], d1mov[c])
                                          for c in range(KP)]
                            pairs.append((wob[mr][:], ops))
                            accum(pr[:], pairs)

                            pz = psB.tile([128, NT], F32, tag="pz", name="pz")
                            pairs = [(trz[mz][:], ohs)]
                            if s == 5:
                                pairs += [(vs[1][:, c], dmov[c])
                                          for c in range(KP)]
                            else:
                                pairs += [(whh_sb[mz][:, c], dmov[c])
                                          for c in range(KP)]
                            if s == 4:
                                pairs += [(vih[1][:, c], d1mov[c])
                                          for c in range(KP)]
                            pairs.append((wob[mz][:], ops))
                            accum(pz[:], pairs)

                            pgh = psC.tile([128, NT], F32, tag="pgh", name="pgh")
                            pairs = [(tn[f][:], ohs)]
                            pairs += [(whh_sb[mn][:, c], dmov[c])
                                      for c in range(KP)]
                            accum(pgh[:], pairs)

                            pg = psC.tile([128, NT], F32, tag="pg", name="pg")
                            if s <= 3:
                                pairs = [(ts1_sb[mn][:], ohs), (wob[mn][:], ops)]
                            elif s == 4:
                                pairs = [(t4gi_sb[f][:], ohs)]
                                pairs += [(vih[2][:, c], d1mov[c])
                                          for c in range(KP)]
                                pairs.append((wob[mn][:], ops))
                            else:
                                pairs = [(t5gi_sb[f][:], ohs)]
                                pairs += [(vs[2][:, c], dmov[c])
                                          for c in range(KP)]
                                pairs.append((wob[mn][:], ops))
                            accum(pg[:], pairs)

                            bcp = None
                            if s < 5:
                                bcp = psB.tile([128, NT], F32, tag="bc",
                                               name="bcp")
                                accum(bcp[:], [(bc_sb[s - 1][f][:], ohs)])
                            gates_tail(b, s, f, jj, pr, pz, pgh, pg, bcp,
                                       hcur[(f, jj)],
                                       {2: "B", 3: "C", 4: "B"}.get(s, ""))

            # software-pipelined block schedule
            sched = [(0, 0), (0, 1)]
            for b in range(nblk):
                sched += [(b, s) for s in (2, 3, 4)]
                if b + 1 < nblk:
                    sched += [(b + 1, 0), (b + 1, 1)]
                sched.append((b, 5))

            for b, s in sched:
                if s == 0:
                    init_block(b)
                elif s == 1:
                    step1(b)
                else:
                    stepn(b, s)

    nc.compile()
    return nc


# ---------------------------------------------------------------------------
# host-side prep / sharding
# ---------------------------------------------------------------------------

def _sig(x):
    return 1.0 / (1.0 + np.exp(-x))


def _q8(x, s=16.0):
    return (np.asarray(x, np.float32) * s).astype(E4M3)


def _2term(x, s=16.0):
    xs = np.asarray(x, np.float32) * s
    A = xs.astype(E4M3)
    B = (xs - A.astype(np.float32)).astype(E4M3)
    return A, B


def _tab_tiles(T):
    """T (nrow_ids=100, C) -> [C/128, 128, 2, 128] fp8 2-term (x16)."""
    C = T.shape[1]
    mt = C // 128
    A, B = _2term(T)
    arr = np.zeros((mt, 128, 2, 128), E4M3)
    # arr[m, p, 0, j] = A[p, 128m+j]
    arr[:, :NEMB, 0, :] = A.T.reshape(mt, 128, NEMB).transpose(0, 2, 1)
    arr[:, :NEMB, 1, :] = B.T.reshape(mt, 128, NEMB).transpose(0, 2, 1)
    return arr


def _wd_tiles(W, mt):
    """W (128mt, 1024) -> [mt, 128, 4, 2, 128] fp8: [m,p,c,i,j]=q8[128m+j,128(2c+i)+p]."""
    Q = _q8(W)
    t = Q.reshape(mt, 128, 4, 2, 128)            # [m, j, c, i, p]
    return np.ascontiguousarray(t.transpose(0, 4, 2, 3, 1))


def _wobs_tiles(Wx):
    """Wx (3072, 64) -> [24, 128, 2, 128] fp8: [[A;A]|[B;B]] packs."""
    A, B = _2term(Wx)
    arr = np.zeros((24, 128, 2, 128), E4M3)
    At = A.reshape(24, 128, 64).transpose(0, 2, 1)   # [m, p, j]
    Bt = B.reshape(24, 128, 64).transpose(0, 2, 1)
    arr[:, :64, 0, :] = At
    arr[:, 64:, 0, :] = At
    arr[:, :64, 1, :] = Bt
    arr[:, 64:, 1, :] = Bt
    return arr


def _prep_shared(emb, W_ih, W_hh, b_ih, b_hh):
    f = np.float32
    W_ih = np.asarray(W_ih, f)
    W_hh = np.asarray(W_hh, f)
    emb = np.asarray(emb, f)
    b_ih = np.asarray(b_ih, f)
    b_hh = np.asarray(b_hh, f)
    Wobs = W_ih[:, :64]
    Wemb = W_ih[:, 64:]
    Wh1 = W_ih[:, :1024]
    Wobs45 = W_ih[:, 1024:1088]
    Wsum = W_hh + Wh1
    br, bz, bn = np.split(b_ih, 3)
    cr, cz, cn = np.split(b_hh, 3)

    # mini-GRU over the 100 ids with obs=0
    giE = emb @ Wemb.T
    hb = np.zeros((NEMB, H), f)
    HBAR = []
    for t in range(3):
        gi, gh = giE, hb @ W_hh.T
        r = _sig(gi[:, :1024] + gh[:, :1024] + br + cr)
        z = _sig(gi[:, 1024:2048] + gh[:, 1024:2048] + bz + cz)
        n = np.tanh(gi[:, 2048:] + bn + r * (gh[:, 2048:] + cn))
        hb = (1 - z) * n + z * hb
        HBAR.append(hb)
    gi, gh = HBAR[0] @ Wh1.T, hb @ W_hh.T
    r = _sig(gi[:, :1024] + gh[:, :1024] + br + cr)
    z = _sig(gi[:, 1024:2048] + gh[:, 1024:2048] + bz + cz)
    n = np.tanh(gi[:, 2048:] + bn + r * (gh[:, 2048:] + cn))
    hb = (1 - z) * n + z * hb
    HBAR.append(hb)

    d = dict(
        whh=_wd_tiles(W_hh, 24),
        wih=_wd_tiles(Wh1, 24),
        wsum=_wd_tiles(Wsum[:2048], 16),
        wobs=_wobs_tiles(Wobs),
        wobs45=_wobs_tiles(Wobs45),
        ts1=_tab_tiles(giE),
        t2rz=_tab_tiles((HBAR[0] @ W_hh.T)[:, :2048] + giE[:, :2048]),
        t2n=_tab_tiles((HBAR[0] @ W_hh.T)[:, 2048:]),
        t3rz=_tab_tiles((HBAR[1] @ W_hh.T)[:, :2048] + giE[:, :2048]),
        t3n=_tab_tiles((HBAR[1] @ W_hh.T)[:, 2048:]),
        t4rz=_tab_tiles((HBAR[2] @ W_hh.T)[:, :2048] + (HBAR[0] @ Wh1.T)[:, :2048]),
        t4n=_tab_tiles((HBAR[2] @ W_hh.T)[:, 2048:]),
        t4gi=_tab_tiles((HBAR[0] @ Wh1.T)[:, 2048:]),
        t5rz=_tab_tiles((HBAR[3] @ Wsum.T)[:, :2048]),
        t5n=_tab_tiles((HBAR[3] @ W_hh.T)[:, 2048:]),
        t5gi=_tab_tiles((HBAR[3] @ Wh1.T)[:, 2048:]),
        bc=np.stack([_tab_tiles(-hbm) for hbm in HBAR]),
        bsum=np.ascontiguousarray((b_ih + b_hh).reshape(24, 128).T, f),
        bih=np.ascontiguousarray(b_ih.reshape(24, 128).T, f),
        bhh256=np.ascontiguousarray((256.0 * b_hh).reshape(24, 128).T, f),
    )
    return d


def _prep_core(buoy_obs, buoy_ids, nbuoy):
    o = np.asarray(buoy_obs, np.float32)
    ids = np.asarray(buoy_ids)
    opk = np.zeros((3, 128, nbuoy), E4M3)
    for t in range(3):
        ot = 16.0 * o[:, t, :].T                     # (64, nb)
        hi = ot.astype(E4M3)
        lo = (ot - hi.astype(np.float32)).astype(E4M3)
        opk[t, :64] = hi
        opk[t, 64:] = lo
    ohm = np.zeros((128, nbuoy), np.float32)
    ohm[ids, np.arange(nbuoy)] = 16.0
    return dict(opk=opk, oh=ohm.astype(E4M3))


_NC_CACHE = {}


def _get_nc(nbuoy, blk):
    key = (nbuoy, blk)
    if key not in _NC_CACHE:
        _NC_CACHE[key] = build(nbuoy, blk)
    return _NC_CACHE[key]


def kernel(buoy_obs, buoy_ids, emb, W_ih, W_hh, b_ih, b_hh):
    buoy_obs = np.asarray(buoy_obs)
    buoy_ids = np.asarray(buoy_ids)
    n = buoy_obs.shape[0]
    per = n // N_CORES
    shared = _prep_shared(emb, W_ih, W_hh, b_ih, b_hh)
    in_maps = []
    for c in range(N_CORES):
        sl = slice(c * per, (c + 1) * per)
        m = dict(shared)
        m.update(_prep_core(buoy_obs[sl], buoy_ids[sl], per))
        in_maps.append(m)

    nc = _get_nc(per, 1024)
    res = run_bass_kernel_spmd(nc, in_maps, list(range(N_CORES)))
    outs = []
    for c in range(N_CORES):
        r = res.results[c]["out_t"]                    # [8, 128, per]
        outs.append(r.transpose(2, 0, 1).reshape(per, H))
    full = np.concatenate(outs, axis=0).astype(np.float32)
    return full[None, :, :]
